# revision 11
# baseline (speedup 1.0000x reference)
"""Distributed attention-layer kernel for 8 TRN2 NeuronCores.

Reference computation (per batch element b):
    Q = Wq @ x[b]; K = Wk @ x[b]; V = Wv @ x[b]
    S = Q^T K  (no scaling);  A = softmax(S, axis=keys)
    out[b] = V @ A^T          # [COUT, N]

Sharding: core i handles (b = i//2, query half h = i%2). The full
attention row block [2048 q x 4096 keys] stays local; no collectives.

Kernel algebra (per core):
    M^T = Wk^T Wq                       (128x128, one matmul)
    Z   = M x[b]   = (M^T)^T x[b]       [128, 4096]
    S^T[m,q] = sum_i Z[i,m] x[i,q]      -> matmul(lhsT=Z_chunk, rhs=xq), f32r
    P = exp(S^T)                        (ScalarE, PSUM->SBUF, bf16 out;
                                         no max-subtraction: max |S| ~ 67)
    num[o,q] = sum_m V^T[m,o] P[m,q]    -> bf16 PSUM-accumulated matmuls
    den[q]   = sum_m P[m,q]             -> P chunks pre-summed on DVE+GpSimd,
                                           then gpsimd partition_all_reduce
                                           (f32 across-partition sum + bcast)
    out = num * (1/den)                 (reciprocal_approx_fast + multiply)

S^T runs in float32r (1 cycle/row at free dim >= 512, ~19-bit mantissa);
the post-exp path runs in bf16 (linear error only; total ~3e-3).
"""

import numpy as np

import concourse.bass as bass
import concourse.bacc as bacc
import concourse.bass_isa as bass_isa
import concourse.mybir as mybir
from concourse.tile import TileContext
from concourse.bass_utils import run_bass_kernel_spmd
from concourse.masks import make_identity

B, CIN, N = 4, 128, 4096
CKEY, COUT = 64, 128
NCORES = 8
NQ = N // 2            # queries per core
QT = 512               # query supertile (PSUM bank width in f32)
NST = NQ // QT         # 4 supertiles
MC = 128               # key-chunk size (partition dim)
NMC = N // MC          # 32 key chunks
GRP = 3                # key chunks per exp group ([128, 1536] = 3 banks)
WIDE_GROUPS = 9        # groups summed wide (chunks 0..26); rest narrow

F32 = mybir.dt.float32
F32R = mybir.dt.float32r
BF16 = mybir.dt.bfloat16
EXP = mybir.ActivationFunctionType.Exp
ADD = mybir.AluOpType.add


def _build() -> bacc.Bacc:
    nc = bacc.Bacc()
    xq = nc.declare_dram_parameter("xq", [CIN, NQ], F32, isOutput=False)
    xk = nc.declare_dram_parameter("xk", [CIN, N], F32, isOutput=False)
    wq = nc.declare_dram_parameter("wq", [CKEY, CIN], F32, isOutput=False)
    wk = nc.declare_dram_parameter("wk", [CKEY, CIN], F32, isOutput=False)
    wv = nc.declare_dram_parameter("wv", [COUT, CIN], F32, isOutput=False)
    out = nc.declare_dram_parameter("out", [COUT, NQ], F32, isOutput=True)

    with TileContext(nc) as tc:
        with (
            tc.tile_pool(name="big", bufs=1) as big,
            tc.tile_pool(name="ptp", bufs=5) as ptp,
            tc.tile_pool(name="accp", bufs=2) as accp,
            tc.tile_pool(name="outp", bufs=2) as outp,
            tc.tile_pool(name="stp", bufs=2, space="PSUM") as stp,
            tc.tile_pool(name="avp", bufs=2, space="PSUM") as avp,
        ):
            # ---- loads (weights first: they gate the Z chain) ----
            wq_sb = big.tile([CKEY, CIN], F32)
            wk_sb = big.tile([CKEY, CIN], F32)
            wv_sb = big.tile([COUT, CIN], F32)
            nc.sync.dma_start(wq_sb[:], wq[:])
            nc.sync.dma_start(wk_sb[:], wk[:])
            nc.sync.dma_start(wv_sb[:], wv[:])
            xk_sb = big.tile([CIN, N], F32)
            NK4 = N // 4
            for qtr in range(4):
                nc.sync.dma_start(xk_sb[:, qtr * NK4:(qtr + 1) * NK4],
                                  xk[:, qtr * NK4:(qtr + 1) * NK4])
            xq_sb = big.tile([CIN, NQ], F32)
            nc.sync.dma_start(xq_sb[:, :NQ // 2], xq[:, :NQ // 2])
            nc.sync.dma_start(xq_sb[:, NQ // 2:], xq[:, NQ // 2:])

            # ---- critical chain first: padded weights -> M^T -> Z ----
            wq_pad = big.tile([CIN, CIN], F32)
            wk_pad = big.tile([CIN, CIN], F32)
            nc.vector.memset(wq_pad[:], 0.0)
            nc.vector.memset(wk_pad[:], 0.0)
            nc.vector.tensor_copy(wq_pad[:CKEY, :], wq_sb[:])
            nc.vector.tensor_copy(wk_pad[:CKEY, :], wk_sb[:])
            wq_r = big.tile([CIN, CIN], F32R)
            wk_r = big.tile([CIN, CIN], F32R)
            nc.vector.tensor_copy(wq_r[:], wq_pad[:])
            nc.vector.tensor_copy(wk_r[:], wk_pad[:])

            mt_ps = stp.tile([CIN, GRP * QT], F32, tag="ps", name="mt_ps")
            nc.tensor.matmul(mt_ps[:, :CIN], wk_r[:], wq_r[:], start=True, stop=True)
            mt_r = big.tile([CIN, CIN], F32R)
            nc.vector.tensor_copy(mt_r[:], mt_ps[:, :CIN])

            # xk_r quarters (DVE) interleaved with Z matmuls below
            xk_r = big.tile([CIN, N], F32R)
            z_r = big.tile([CIN, N], F32R)
            for qtr in range(4):
                sl = slice(qtr * NK4, (qtr + 1) * NK4)
                nc.vector.tensor_copy(xk_r[:, sl], xk_sb[:, sl])
                zp = stp.tile([CIN, GRP * QT], F32, tag="ps", name="zp")
                nc.tensor.matmul(zp[:, :QT], mt_r[:],
                                 xk_r[:, qtr * NK4: qtr * NK4 + QT],
                                 start=True, stop=True)
                nc.tensor.matmul(zp[:, QT: 2 * QT], mt_r[:],
                                 xk_r[:, qtr * NK4 + QT: (qtr + 1) * NK4],
                                 start=True, stop=True)
                nc.vector.tensor_copy(z_r[:, sl], zp[:, : 2 * QT])

            # queries (first half gates supertile 0)
            xq_r = big.tile([CIN, NQ], F32R)
            nc.vector.tensor_copy(xq_r[:, :NQ // 2], xq_sb[:, :NQ // 2])
            nc.vector.tensor_copy(xq_r[:, NQ // 2:], xq_sb[:, NQ // 2:])

            # ---- V^T path (bf16), off the critical chain ----
            xk_bf = big.tile([CIN, N], BF16)
            for qtr in range(4):
                sl = slice(qtr * NK4, (qtr + 1) * NK4)
                eng = nc.gpsimd if qtr % 2 == 0 else nc.vector
                eng.tensor_copy(xk_bf[:, sl], xk_sb[:, sl])
            wv_r = big.tile([COUT, CIN], F32R)
            nc.vector.tensor_copy(wv_r[:], wv_sb[:])
            ident_f = big.tile([CIN, CIN], F32)
            make_identity(nc, ident_f[:])
            ident_r = big.tile([CIN, CIN], F32R)
            nc.vector.tensor_copy(ident_r[:], ident_f[:])
            wvt_ps = stp.tile([CIN, GRP * QT], F32, tag="ps", name="wvt_ps")
            nc.tensor.matmul(wvt_ps[:, :CIN], wv_r[:], ident_r[:], start=True, stop=True)
            wvt_bf = big.tile([CIN, COUT], BF16)
            nc.vector.tensor_copy(wvt_bf[:], wvt_ps[:, :CIN])

            vt_bf = big.tile([CIN, NMC, MC], BF16)
            for grp in range(NMC // 8):
                vp = stp.tile([CIN, GRP * QT], F32, tag="ps", name="vp")
                for k in range(8):
                    c = grp * 8 + k
                    nc.tensor.matmul(
                        vp[:, k * MC: (k + 1) * MC],
                        xk_bf[:, c * MC: (c + 1) * MC],
                        wvt_bf[:],
                        start=True, stop=True,
                    )
                nc.vector.tensor_copy(vt_bf[:, grp * 8: (grp + 1) * 8, :],
                                      vp[:, : 8 * MC])

            ones_col_f = big.tile([CIN, 1], F32)
            nc.vector.memset(ones_col_f[:], 1.0)
            ones_col = big.tile([CIN, 1], BF16)
            nc.vector.tensor_copy(ones_col[:], ones_col_f[:])

            # ---- main loop over query supertiles ----
            groups = []
            c = 0
            while c < NMC:
                cnt = min(GRP, NMC - c)
                groups.append((c, cnt))
                c += cnt
            n_wide = WIDE_GROUPS * GRP  # chunks summed via wide group adds

            for st in range(NST):
                q0 = st * QT
                xq_st = xq_r[:, q0: q0 + QT]
                av = avp.tile([COUT, QT], F32, tag="av", name="av")
                acc_e = accp.tile([MC, GRP * QT], BF16, name="acc_e", tag="acc_e")
                acc_o = accp.tile([MC, GRP * QT], BF16, name="acc_o", tag="acc_o")
                accn = accp.tile([MC, QT], BF16, name="accn", tag="accn")
                seen = [0, 0]
                for gi, (c0, cnt) in enumerate(groups):
                    ps = stp.tile([MC, GRP * QT], F32, tag="ps", name="ps")
                    for k in range(cnt):
                        nc.tensor.matmul(
                            ps[:, k * QT: (k + 1) * QT],
                            z_r[:, (c0 + k) * MC: (c0 + k + 1) * MC],
                            xq_st, start=True, stop=True)
                    pt = ptp.tile([MC, GRP * QT], BF16, tag="pt", name="pt")
                    nc.scalar.activation(pt[:, : cnt * QT], ps[:, : cnt * QT], EXP)
                    for k in range(cnt):
                        cc = c0 + k
                        nc.tensor.matmul(av[:], vt_bf[:, cc, :],
                                         pt[:, k * QT: (k + 1) * QT],
                                         start=(cc == 0), stop=(cc == NMC - 1))
                    # ---- den accumulation ----
                    if gi < WIDE_GROUPS:
                        # whole-group wide adds, alternating GpSimd / DVE
                        par = gi % 2
                        eng = nc.gpsimd if par == 0 else nc.vector
                        acc = acc_e if par == 0 else acc_o
                        if seen[par] == 0:
                            eng.tensor_copy(acc[:], pt[:])
                        else:
                            eng.tensor_tensor(acc[:], acc[:], pt[:], ADD)
                        seen[par] += 1
                    else:
                        if gi == WIDE_GROUPS:
                            # combine + fold the wide accumulators down to
                            # [128, QT] (runs while the tail groups compute)
                            nc.vector.tensor_tensor(acc_e[:], acc_e[:], acc_o[:], ADD)
                            nc.vector.tensor_tensor(
                                acc_e[:, :QT], acc_e[:, :QT],
                                acc_e[:, QT: 2 * QT], ADD)
                            nc.vector.tensor_tensor(
                                accn[:], acc_e[:, :QT],
                                acc_e[:, 2 * QT: 3 * QT], ADD)
                        for k in range(cnt):
                            nc.vector.tensor_tensor(
                                accn[:], accn[:],
                                pt[:, k * QT: (k + 1) * QT], ADD)

                # den: f32 sum over partitions, broadcast to all partitions
                den_b = outp.tile([MC, QT], F32, name="den_b")
                nc.gpsimd.partition_all_reduce(den_b[:], accn[:], MC,
                                               bass_isa.ReduceOp.add)
                rb_sb = outp.tile([COUT, QT], F32, name="rb_sb")
                nc.vector.reciprocal_approx_fast(rb_sb[:], den_b[:])
                o_sb = outp.tile([COUT, QT], F32, name="o_sb")
                nc.vector.tensor_tensor(o_sb[:], av[:], rb_sb[:],
                                        mybir.AluOpType.mult)
                nc.sync.dma_start(out[:, q0: q0 + QT], o_sb[:])

    nc.finalize()
    return nc


_NC_CACHE: list = []
LAST_RESULTS = None


def _get_nc() -> bacc.Bacc:
    if not _NC_CACHE:
        _NC_CACHE.append(_build())
    return _NC_CACHE[0]


def kernel(x, Wq, Wk, Wv, _trace=False):
    global LAST_RESULTS
    x = np.asarray(x, dtype=np.float32)
    wq = np.ascontiguousarray(np.asarray(Wq, dtype=np.float32))
    wk = np.ascontiguousarray(np.asarray(Wk, dtype=np.float32))
    wv = np.ascontiguousarray(np.asarray(Wv, dtype=np.float32))

    nc = _get_nc()
    in_maps = []
    for i in range(NCORES):
        b, h = divmod(i, 2)
        in_maps.append({
            "xq": np.ascontiguousarray(x[b][:, h * NQ: (h + 1) * NQ]),
            "xk": np.ascontiguousarray(x[b]),
            "wq": wq,
            "wk": wk,
            "wv": wv,
        })
    res = run_bass_kernel_spmd(nc, in_maps, core_ids=list(range(NCORES)),
                               trace=_trace)
    LAST_RESULTS = res
    out = np.empty((B, COUT, N), dtype=np.float32)
    for i in range(NCORES):
        b, h = divmod(i, 2)
        out[b][:, h * NQ: (h + 1) * NQ] = res.results[i]["out"]
    return out


# revision 13
# speedup vs baseline: 1.8020x; 1.8020x over previous
"""Distributed attention-layer kernel for 8 TRN2 NeuronCores.

Reference computation (per batch element b):
    Q = Wq @ x[b]; K = Wk @ x[b]; V = Wv @ x[b]
    S = Q^T K  (no scaling);  A = softmax(S, axis=keys)
    out[b] = V @ A^T          # [COUT, N]

Sharding: core i handles (b = i//2, query half h = i%2). The full
attention row block [2048 q x 4096 keys] stays local; no collectives.

Kernel algebra (per core):
    M^T = Wk^T Wq                       (128x128, one matmul)
    Z   = M x[b]   = (M^T)^T x[b]       [128, 4096]
    S^T[m,q] = sum_i Z[i,m] x[i,q]      -> matmul(lhsT=Z_chunk, rhs=xq), f32r
    P = exp(S^T)                        (ScalarE, PSUM->SBUF, bf16 out;
                                         no max-subtraction: max |S| ~ 67)
    num[o,q] = sum_m V^T[m,o] P[m,q]    -> bf16 PSUM-accumulated matmuls
    den[q]   = sum_m P[m,q]             -> P chunks pre-summed on DVE+GpSimd,
                                           then gpsimd partition_all_reduce
                                           (f32 across-partition sum + bcast)
    out = num * (1/den)                 (reciprocal_approx_fast + multiply)

S^T runs in float32r (1 cycle/row at free dim >= 512, ~19-bit mantissa);
the post-exp path runs in bf16 (linear error only; total ~3e-3).
"""

import numpy as np

import concourse.bass as bass
import concourse.bacc as bacc
import concourse.bass_isa as bass_isa
import concourse.mybir as mybir
from concourse.tile import TileContext
from concourse.bass_utils import run_bass_kernel_spmd
from concourse.masks import make_identity

B, CIN, N = 4, 128, 4096
CKEY, COUT = 64, 128
NCORES = 8
NQ = N // 2            # queries per core
QT = 512               # query supertile (PSUM bank width in f32)
NST = NQ // QT         # 4 supertiles
MC = 128               # key-chunk size (partition dim)
NMC = N // MC          # 32 key chunks
GRP = 3                # key chunks per exp group ([128, 1536] = 3 banks)
WIDE_GROUPS = 9        # groups summed wide (chunks 0..26); rest narrow

F32 = mybir.dt.float32
F32R = mybir.dt.float32r
BF16 = mybir.dt.bfloat16
EXP = mybir.ActivationFunctionType.Exp
ADD = mybir.AluOpType.add


def _build() -> bacc.Bacc:
    nc = bacc.Bacc()
    xq = nc.declare_dram_parameter("xq", [CIN, NQ], F32, isOutput=False)
    xk = nc.declare_dram_parameter("xk", [CIN, N], F32, isOutput=False)
    wq = nc.declare_dram_parameter("wq", [CKEY, CIN], F32, isOutput=False)
    wk = nc.declare_dram_parameter("wk", [CKEY, CIN], F32, isOutput=False)
    wv = nc.declare_dram_parameter("wv", [COUT, CIN], F32, isOutput=False)
    out = nc.declare_dram_parameter("out", [COUT, NQ], F32, isOutput=True)

    with TileContext(nc) as tc:
        with (
            tc.tile_pool(name="big", bufs=1) as big,
            tc.tile_pool(name="ptp", bufs=5) as ptp,
            tc.tile_pool(name="accp", bufs=2) as accp,
            tc.tile_pool(name="outp", bufs=2) as outp,
            tc.tile_pool(name="stp", bufs=2, space="PSUM") as stp,
            tc.tile_pool(name="avp", bufs=2, space="PSUM") as avp,
        ):
            # ---- loads (weights first: they gate the Z chain) ----
            wq_sb = big.tile([CKEY, CIN], F32)
            wk_sb = big.tile([CKEY, CIN], F32)
            wv_sb = big.tile([COUT, CIN], F32)
            nc.sync.dma_start(wq_sb[:], wq[:])
            nc.sync.dma_start(wk_sb[:], wk[:])
            nc.sync.dma_start(wv_sb[:], wv[:])
            xk_sb = big.tile([CIN, N], F32)
            NK4 = N // 4
            for qtr in range(4):
                nc.sync.dma_start(xk_sb[:, qtr * NK4:(qtr + 1) * NK4],
                                  xk[:, qtr * NK4:(qtr + 1) * NK4])
            xq_sb = big.tile([CIN, NQ], F32)
            nc.sync.dma_start(xq_sb[:, :NQ // 2], xq[:, :NQ // 2])
            nc.sync.dma_start(xq_sb[:, NQ // 2:], xq[:, NQ // 2:])

            # ---- critical chain first: padded weights -> M^T -> Z ----
            wq_pad = big.tile([CIN, CIN], F32)
            wk_pad = big.tile([CIN, CIN], F32)
            nc.vector.memset(wq_pad[:], 0.0)
            nc.vector.memset(wk_pad[:], 0.0)
            nc.vector.tensor_copy(wq_pad[:CKEY, :], wq_sb[:])
            nc.vector.tensor_copy(wk_pad[:CKEY, :], wk_sb[:])
            wq_r = big.tile([CIN, CIN], F32R)
            wk_r = big.tile([CIN, CIN], F32R)
            nc.vector.tensor_copy(wq_r[:], wq_pad[:])
            nc.vector.tensor_copy(wk_r[:], wk_pad[:])

            mt_ps = stp.tile([CIN, GRP * QT], F32, tag="ps", name="mt_ps")
            nc.tensor.matmul(mt_ps[:, :CIN], wk_r[:], wq_r[:], start=True, stop=True)
            mt_r = big.tile([CIN, CIN], F32R)
            nc.vector.tensor_copy(mt_r[:], mt_ps[:, :CIN])

            # xk_r quarters (DVE) interleaved with Z matmuls below
            xk_r = big.tile([CIN, N], F32R)
            z_r = big.tile([CIN, N], F32R)
            for qtr in range(4):
                sl = slice(qtr * NK4, (qtr + 1) * NK4)
                nc.vector.tensor_copy(xk_r[:, sl], xk_sb[:, sl])
                zp = stp.tile([CIN, GRP * QT], F32, tag="ps", name="zp")
                nc.tensor.matmul(zp[:, :QT], mt_r[:],
                                 xk_r[:, qtr * NK4: qtr * NK4 + QT],
                                 start=True, stop=True)
                nc.tensor.matmul(zp[:, QT: 2 * QT], mt_r[:],
                                 xk_r[:, qtr * NK4 + QT: (qtr + 1) * NK4],
                                 start=True, stop=True)
                nc.vector.tensor_copy(z_r[:, sl], zp[:, : 2 * QT])

            # queries (first half gates supertile 0)
            xq_r = big.tile([CIN, NQ], F32R)
            nc.vector.tensor_copy(xq_r[:, :NQ // 2], xq_sb[:, :NQ // 2])
            nc.vector.tensor_copy(xq_r[:, NQ // 2:], xq_sb[:, NQ // 2:])

            # ---- V^T path (bf16), off the critical chain ----
            xk_bf = big.tile([CIN, N], BF16)
            for qtr in range(4):
                sl = slice(qtr * NK4, (qtr + 1) * NK4)
                nc.vector.tensor_copy(xk_bf[:, sl], xk_sb[:, sl])
            wv_r = big.tile([COUT, CIN], F32R)
            nc.vector.tensor_copy(wv_r[:], wv_sb[:])
            ident_f = big.tile([CIN, CIN], F32)
            make_identity(nc, ident_f[:])
            ident_r = big.tile([CIN, CIN], F32R)
            nc.vector.tensor_copy(ident_r[:], ident_f[:])
            wvt_ps = stp.tile([CIN, GRP * QT], F32, tag="ps", name="wvt_ps")
            nc.tensor.matmul(wvt_ps[:, :CIN], wv_r[:], ident_r[:], start=True, stop=True)
            wvt_bf = big.tile([CIN, COUT], BF16)
            nc.vector.tensor_copy(wvt_bf[:], wvt_ps[:, :CIN])

            vt_bf = big.tile([CIN, NMC, MC], BF16)
            for grp in range(NMC // 8):
                vp = stp.tile([CIN, GRP * QT], F32, tag="ps", name="vp")
                for k in range(8):
                    c = grp * 8 + k
                    nc.tensor.matmul(
                        vp[:, k * MC: (k + 1) * MC],
                        xk_bf[:, c * MC: (c + 1) * MC],
                        wvt_bf[:],
                        start=True, stop=True,
                    )
                nc.vector.tensor_copy(vt_bf[:, grp * 8: (grp + 1) * 8, :],
                                      vp[:, : 8 * MC])

            ones_col_f = big.tile([CIN, 1], F32)
            nc.vector.memset(ones_col_f[:], 1.0)
            ones_col = big.tile([CIN, 1], BF16)
            nc.vector.tensor_copy(ones_col[:], ones_col_f[:])

            # ---- main loop over query supertiles ----
            groups = []
            c = 0
            while c < NMC:
                cnt = min(GRP, NMC - c)
                groups.append((c, cnt))
                c += cnt
            n_wide = WIDE_GROUPS * GRP  # chunks summed via wide group adds

            for st in range(NST):
                q0 = st * QT
                xq_st = xq_r[:, q0: q0 + QT]
                av = avp.tile([COUT, QT], F32, tag="av", name="av")
                acc_e = accp.tile([MC, GRP * QT], BF16, name="acc_e", tag="acc_e")
                acc_o = accp.tile([MC, GRP * QT], BF16, name="acc_o", tag="acc_o")
                accn = accp.tile([MC, QT], BF16, name="accn", tag="accn")
                seen = [0, 0]
                for gi, (c0, cnt) in enumerate(groups):
                    ps = stp.tile([MC, GRP * QT], F32, tag="ps", name="ps")
                    for k in range(cnt):
                        nc.tensor.matmul(
                            ps[:, k * QT: (k + 1) * QT],
                            z_r[:, (c0 + k) * MC: (c0 + k + 1) * MC],
                            xq_st, start=True, stop=True)
                    pt = ptp.tile([MC, GRP * QT], BF16, tag="pt", name="pt")
                    nc.scalar.activation(pt[:, : cnt * QT], ps[:, : cnt * QT], EXP)
                    for k in range(cnt):
                        cc = c0 + k
                        nc.tensor.matmul(av[:], vt_bf[:, cc, :],
                                         pt[:, k * QT: (k + 1) * QT],
                                         start=(cc == 0), stop=(cc == NMC - 1))
                    # ---- den accumulation ----
                    if gi < WIDE_GROUPS:
                        # whole-group wide adds on DVE (GpSimd 2-input ops
                        # measured ~4x slower; PE is the bottleneck anyway)
                        par = gi % 2
                        acc = acc_e if par == 0 else acc_o
                        if seen[par] == 0:
                            nc.vector.tensor_copy(acc[:], pt[:])
                        else:
                            nc.vector.tensor_tensor(acc[:], acc[:], pt[:], ADD)
                        seen[par] += 1
                    else:
                        if gi == WIDE_GROUPS:
                            # combine + fold the wide accumulators down to
                            # [128, QT] (runs while the tail groups compute)
                            nc.vector.tensor_tensor(acc_e[:], acc_e[:], acc_o[:], ADD)
                            nc.vector.tensor_tensor(
                                acc_e[:, :QT], acc_e[:, :QT],
                                acc_e[:, QT: 2 * QT], ADD)
                            nc.vector.tensor_tensor(
                                accn[:], acc_e[:, :QT],
                                acc_e[:, 2 * QT: 3 * QT], ADD)
                        for k in range(cnt):
                            nc.vector.tensor_tensor(
                                accn[:], accn[:],
                                pt[:, k * QT: (k + 1) * QT], ADD)

                # den: f32 sum over partitions, broadcast to all partitions
                den_b = outp.tile([MC, QT], F32, name="den_b")
                nc.gpsimd.partition_all_reduce(den_b[:], accn[:], MC,
                                               bass_isa.ReduceOp.add)
                rb_sb = outp.tile([COUT, QT], F32, name="rb_sb")
                nc.vector.reciprocal_approx_fast(rb_sb[:], den_b[:])
                o_sb = outp.tile([COUT, QT], F32, name="o_sb")
                nc.vector.tensor_tensor(o_sb[:], av[:], rb_sb[:],
                                        mybir.AluOpType.mult)
                nc.sync.dma_start(out[:, q0: q0 + QT], o_sb[:])

    nc.finalize()
    return nc


_NC_CACHE: list = []
LAST_RESULTS = None


def _get_nc() -> bacc.Bacc:
    if not _NC_CACHE:
        _NC_CACHE.append(_build())
    return _NC_CACHE[0]


def kernel(x, Wq, Wk, Wv, _trace=False):
    global LAST_RESULTS
    x = np.asarray(x, dtype=np.float32)
    wq = np.ascontiguousarray(np.asarray(Wq, dtype=np.float32))
    wk = np.ascontiguousarray(np.asarray(Wk, dtype=np.float32))
    wv = np.ascontiguousarray(np.asarray(Wv, dtype=np.float32))

    nc = _get_nc()
    in_maps = []
    for i in range(NCORES):
        b, h = divmod(i, 2)
        in_maps.append({
            "xq": np.ascontiguousarray(x[b][:, h * NQ: (h + 1) * NQ]),
            "xk": np.ascontiguousarray(x[b]),
            "wq": wq,
            "wk": wk,
            "wv": wv,
        })
    res = run_bass_kernel_spmd(nc, in_maps, core_ids=list(range(NCORES)),
                               trace=_trace)
    LAST_RESULTS = res
    out = np.empty((B, COUT, N), dtype=np.float32)
    for i in range(NCORES):
        b, h = divmod(i, 2)
        out[b][:, h * NQ: (h + 1) * NQ] = res.results[i]["out"]
    return out


# revision 16
# speedup vs baseline: 1.9156x; 1.0630x over previous
"""Distributed attention-layer kernel for 8 TRN2 NeuronCores.

Reference computation (per batch element b):
    Q = Wq @ x[b]; K = Wk @ x[b]; V = Wv @ x[b]
    S = Q^T K  (no scaling);  A = softmax(S, axis=keys)
    out[b] = V @ A^T          # [COUT, N]

Sharding: core i handles (b = i//2, query half h = i%2). The full
attention row block [2048 q x 4096 keys] stays local; no collectives.

Kernel algebra (per core):
    M^T = Wk^T Wq                       (128x128, one matmul)
    Z   = M x[b]   = (M^T)^T x[b]       [128, 4096]
    S^T[m,q] = sum_i Z[i,m] x[i,q]      -> matmul(lhsT=Z_chunk, rhs=xq), f32r
    P = exp(S^T)                        (ScalarE, PSUM->SBUF, bf16 out;
                                         no max-subtraction: max |S| ~ 67)
    num[o,q] = sum_m V^T[m,o] P[m,q]    -> bf16 PSUM-accumulated matmuls
    den[q]   = sum_m P[m,q]             -> P chunks pre-summed on DVE+GpSimd,
                                           then gpsimd partition_all_reduce
                                           (f32 across-partition sum + bcast)
    out = num * (1/den)                 (reciprocal_approx_fast + multiply)

S^T runs in float32r (1 cycle/row at free dim >= 512, ~19-bit mantissa);
the post-exp path runs in bf16 (linear error only; total ~3e-3).
"""

import numpy as np

import concourse.bass as bass
import concourse.bacc as bacc
import concourse.bass_isa as bass_isa
import concourse.mybir as mybir
from concourse.tile import TileContext
from concourse.bass_utils import run_bass_kernel_spmd
from concourse.masks import make_identity

B, CIN, N = 4, 128, 4096
CKEY, COUT = 64, 128
NCORES = 8
NQ = N // 2            # queries per core
QT = 512               # query supertile (PSUM bank width in f32)
NST = NQ // QT         # 4 supertiles
MC = 128               # key-chunk size (partition dim)
NMC = N // MC          # 32 key chunks
GRP = 3                # key chunks per exp group ([128, 1536] = 3 banks)
WIDE_GROUPS = 9        # groups summed wide (chunks 0..26); rest narrow

F32 = mybir.dt.float32
F32R = mybir.dt.float32r
BF16 = mybir.dt.bfloat16
EXP = mybir.ActivationFunctionType.Exp
ADD = mybir.AluOpType.add


def _build() -> bacc.Bacc:
    nc = bacc.Bacc()
    xq = nc.declare_dram_parameter("xq", [CIN, NQ], F32, isOutput=False)
    xk = nc.declare_dram_parameter("xk", [CIN, N], F32, isOutput=False)
    wq = nc.declare_dram_parameter("wq", [CKEY, CIN], F32, isOutput=False)
    wk = nc.declare_dram_parameter("wk", [CKEY, CIN], F32, isOutput=False)
    wv = nc.declare_dram_parameter("wv", [COUT, CIN], F32, isOutput=False)
    out = nc.declare_dram_parameter("out", [COUT, NQ], F32, isOutput=True)

    with TileContext(nc) as tc:
        with (
            tc.tile_pool(name="big", bufs=1) as big,
            tc.tile_pool(name="ptp", bufs=5) as ptp,
            tc.tile_pool(name="accp", bufs=2) as accp,
            tc.tile_pool(name="outp", bufs=2) as outp,
            tc.tile_pool(name="stp", bufs=2, space="PSUM") as stp,
            tc.tile_pool(name="avp", bufs=2, space="PSUM") as avp,
        ):
            # ---- loads (weights first: they gate the Z chain) ----
            wq_sb = big.tile([CKEY, CIN], F32)
            wk_sb = big.tile([CKEY, CIN], F32)
            wv_sb = big.tile([COUT, CIN], F32)
            nc.sync.dma_start(wq_sb[:], wq[:])
            nc.sync.dma_start(wk_sb[:], wk[:])
            nc.sync.dma_start(wv_sb[:], wv[:])
            xk_sb = big.tile([CIN, N], F32)
            NK4 = N // 4
            for qtr in range(4):
                nc.sync.dma_start(xk_sb[:, qtr * NK4:(qtr + 1) * NK4],
                                  xk[:, qtr * NK4:(qtr + 1) * NK4])
            xq_sb = big.tile([CIN, NQ], F32)
            nc.sync.dma_start(xq_sb[:, :NQ // 2], xq[:, :NQ // 2])
            nc.sync.dma_start(xq_sb[:, NQ // 2:], xq[:, NQ // 2:])

            # ---- critical chain first: padded weights -> M^T -> Z ----
            wq_pad = big.tile([CIN, CIN], F32)
            wk_pad = big.tile([CIN, CIN], F32)
            nc.vector.memset(wq_pad[:], 0.0)
            nc.vector.memset(wk_pad[:], 0.0)
            nc.vector.tensor_copy(wq_pad[:CKEY, :], wq_sb[:])
            nc.vector.tensor_copy(wk_pad[:CKEY, :], wk_sb[:])
            wq_r = big.tile([CIN, CIN], F32R)
            wk_r = big.tile([CIN, CIN], F32R)
            nc.vector.tensor_copy(wq_r[:], wq_pad[:])
            nc.vector.tensor_copy(wk_r[:], wk_pad[:])

            mt_ps = stp.tile([CIN, GRP * QT], F32, tag="ps", name="mt_ps")
            nc.tensor.matmul(mt_ps[:, :CIN], wk_r[:], wq_r[:], start=True, stop=True)
            mt_r = big.tile([CIN, CIN], F32R)
            nc.vector.tensor_copy(mt_r[:], mt_ps[:, :CIN])

            # xk_r quarters (DVE back-to-back) with Z matmuls; Z psum
            # copy-backs go to ScalarE so they don't serialize the casts
            xk_r = big.tile([CIN, N], F32R)
            z_r = big.tile([CIN, N], F32R)
            for qtr in range(4):
                sl = slice(qtr * NK4, (qtr + 1) * NK4)
                nc.vector.tensor_copy(xk_r[:, sl], xk_sb[:, sl])
                zp = stp.tile([CIN, GRP * QT], F32, tag="ps", name="zp")
                nc.tensor.matmul(zp[:, :QT], mt_r[:],
                                 xk_r[:, qtr * NK4: qtr * NK4 + QT],
                                 start=True, stop=True)
                nc.tensor.matmul(zp[:, QT: 2 * QT], mt_r[:],
                                 xk_r[:, qtr * NK4 + QT: (qtr + 1) * NK4],
                                 start=True, stop=True)
                nc.scalar.copy(z_r[:, sl], zp[:, : 2 * QT])

            # queries (first half gates supertile 0)
            xq_r = big.tile([CIN, NQ], F32R)
            nc.vector.tensor_copy(xq_r[:, :NQ // 2], xq_sb[:, :NQ // 2])
            nc.vector.tensor_copy(xq_r[:, NQ // 2:], xq_sb[:, NQ // 2:])

            # ---- V^T path (bf16), off the critical chain ----
            xk_bf = big.tile([CIN, N], BF16)
            for qtr in range(4):
                sl = slice(qtr * NK4, (qtr + 1) * NK4)
                nc.vector.tensor_copy(xk_bf[:, sl], xk_sb[:, sl])
            wv_r = big.tile([COUT, CIN], F32R)
            nc.vector.tensor_copy(wv_r[:], wv_sb[:])
            ident_f = big.tile([CIN, CIN], F32)
            make_identity(nc, ident_f[:])
            ident_r = big.tile([CIN, CIN], F32R)
            nc.vector.tensor_copy(ident_r[:], ident_f[:])
            wvt_ps = stp.tile([CIN, GRP * QT], F32, tag="ps", name="wvt_ps")
            nc.tensor.matmul(wvt_ps[:, :CIN], wv_r[:], ident_r[:], start=True, stop=True)
            wvt_bf = big.tile([CIN, COUT], BF16)
            nc.vector.tensor_copy(wvt_bf[:], wvt_ps[:, :CIN])

            vt_bf = big.tile([CIN, NMC, MC], BF16)
            for grp in range(NMC // 8):
                vp = stp.tile([CIN, GRP * QT], F32, tag="ps", name="vp")
                for k in range(8):
                    c = grp * 8 + k
                    nc.tensor.matmul(
                        vp[:, k * MC: (k + 1) * MC],
                        xk_bf[:, c * MC: (c + 1) * MC],
                        wvt_bf[:],
                        start=True, stop=True,
                    )
                nc.vector.tensor_copy(vt_bf[:, grp * 8: (grp + 1) * 8, :],
                                      vp[:, : 8 * MC])

            ones_col_f = big.tile([CIN, 1], F32)
            nc.vector.memset(ones_col_f[:], 1.0)
            ones_col = big.tile([CIN, 1], BF16)
            nc.vector.tensor_copy(ones_col[:], ones_col_f[:])
            ones_row_f = big.tile([1, CIN], F32)
            nc.vector.memset(ones_row_f[:], 1.0)
            ones_row = big.tile([1, CIN], F32R)
            nc.vector.tensor_copy(ones_row[:], ones_row_f[:])

            # ---- main loop over query supertiles ----
            groups = []
            c = 0
            while c < NMC:
                cnt = min(GRP, NMC - c)
                groups.append((c, cnt))
                c += cnt
            n_wide = WIDE_GROUPS * GRP  # chunks summed via wide group adds

            for st in range(NST):
                q0 = st * QT
                xq_st = xq_r[:, q0: q0 + QT]
                av = avp.tile([COUT, QT], F32, tag="av", name="av")
                acc_e = accp.tile([MC, GRP * QT], BF16, name="acc_e", tag="acc_e")
                acc_o = accp.tile([MC, GRP * QT], BF16, name="acc_o", tag="acc_o")
                accn = accp.tile([MC, QT], BF16, name="accn", tag="accn")
                seen = [0, 0]
                for gi, (c0, cnt) in enumerate(groups):
                    ps = stp.tile([MC, GRP * QT], F32, tag="ps", name="ps")
                    for k in range(cnt):
                        nc.tensor.matmul(
                            ps[:, k * QT: (k + 1) * QT],
                            z_r[:, (c0 + k) * MC: (c0 + k + 1) * MC],
                            xq_st, start=True, stop=True)
                    pt = ptp.tile([MC, GRP * QT], BF16, tag="pt", name="pt")
                    nc.scalar.activation(pt[:, : cnt * QT], ps[:, : cnt * QT], EXP)
                    for k in range(cnt):
                        cc = c0 + k
                        nc.tensor.matmul(av[:], vt_bf[:, cc, :],
                                         pt[:, k * QT: (k + 1) * QT],
                                         start=(cc == 0), stop=(cc == NMC - 1))
                    # ---- den accumulation ----
                    if gi < WIDE_GROUPS:
                        # whole-group wide adds on DVE (GpSimd 2-input ops
                        # measured ~4x slower; PE is the bottleneck anyway)
                        par = gi % 2
                        acc = acc_e if par == 0 else acc_o
                        if seen[par] == 0:
                            nc.vector.tensor_copy(acc[:], pt[:])
                        else:
                            nc.vector.tensor_tensor(acc[:], acc[:], pt[:], ADD)
                        seen[par] += 1
                    else:
                        if gi == WIDE_GROUPS:
                            # combine + fold the wide accumulators down to
                            # [128, QT] (runs while the tail groups compute)
                            nc.vector.tensor_tensor(acc_e[:], acc_e[:], acc_o[:], ADD)
                            nc.vector.tensor_tensor(
                                acc_e[:, :QT], acc_e[:, :QT],
                                acc_e[:, QT: 2 * QT], ADD)
                            nc.vector.tensor_tensor(
                                accn[:], acc_e[:, :QT],
                                acc_e[:, 2 * QT: 3 * QT], ADD)
                        for k in range(cnt):
                            nc.vector.tensor_tensor(
                                accn[:], accn[:],
                                pt[:, k * QT: (k + 1) * QT], ADD)

                rb_sb = outp.tile([COUT, QT], F32, name="rb_sb")
                if st < NST - 1:
                    # den: f32 sum over partitions + broadcast on GpSimd
                    # (slow but fully overlapped with the next supertile)
                    den_b = outp.tile([MC, QT], F32, name="den_b")
                    nc.gpsimd.partition_all_reduce(den_b[:], accn[:], MC,
                                                   bass_isa.ReduceOp.add)
                    nc.vector.reciprocal_approx_fast(rb_sb[:], den_b[:])
                else:
                    # last supertile: nothing left to overlap with, so use
                    # the faster PE path through freed PSUM slots
                    dn_ps = stp.tile([MC, GRP * QT], F32, tag="ps", name="dn_ps")
                    nc.tensor.matmul(dn_ps[:1, :QT], ones_col[:], accn[:],
                                     start=True, stop=True)
                    den_r = outp.tile([1, QT], F32R, name="den_r")
                    nc.vector.tensor_copy(den_r[:], dn_ps[:1, :QT])
                    rb_ps = stp.tile([MC, GRP * QT], F32, tag="ps", name="rb_ps")
                    nc.tensor.matmul(rb_ps[:, :QT], ones_row[:], den_r[:],
                                     start=True, stop=True)
                    nc.vector.reciprocal_approx_fast(rb_sb[:], rb_ps[:, :QT])
                o_sb = outp.tile([COUT, QT], F32, name="o_sb")
                nc.vector.tensor_tensor(o_sb[:], av[:], rb_sb[:],
                                        mybir.AluOpType.mult)
                nc.sync.dma_start(out[:, q0: q0 + QT], o_sb[:])

    nc.finalize()
    return nc


_NC_CACHE: list = []
LAST_RESULTS = None


def _get_nc() -> bacc.Bacc:
    if not _NC_CACHE:
        _NC_CACHE.append(_build())
    return _NC_CACHE[0]


def kernel(x, Wq, Wk, Wv, _trace=False):
    global LAST_RESULTS
    x = np.asarray(x, dtype=np.float32)
    wq = np.ascontiguousarray(np.asarray(Wq, dtype=np.float32))
    wk = np.ascontiguousarray(np.asarray(Wk, dtype=np.float32))
    wv = np.ascontiguousarray(np.asarray(Wv, dtype=np.float32))

    nc = _get_nc()
    in_maps = []
    for i in range(NCORES):
        b, h = divmod(i, 2)
        in_maps.append({
            "xq": np.ascontiguousarray(x[b][:, h * NQ: (h + 1) * NQ]),
            "xk": np.ascontiguousarray(x[b]),
            "wq": wq,
            "wk": wk,
            "wv": wv,
        })
    res = run_bass_kernel_spmd(nc, in_maps, core_ids=list(range(NCORES)),
                               trace=_trace)
    LAST_RESULTS = res
    out = np.empty((B, COUT, N), dtype=np.float32)
    for i in range(NCORES):
        b, h = divmod(i, 2)
        out[b][:, h * NQ: (h + 1) * NQ] = res.results[i]["out"]
    return out


# revision 18
# speedup vs baseline: 1.9759x; 1.0315x over previous
"""Distributed attention-layer kernel for 8 TRN2 NeuronCores.

Reference computation (per batch element b):
    Q = Wq @ x[b]; K = Wk @ x[b]; V = Wv @ x[b]
    S = Q^T K  (no scaling);  A = softmax(S, axis=keys)
    out[b] = V @ A^T          # [COUT, N]

Sharding: core i handles (b = i//2, query half h = i%2). The full
attention row block [2048 q x 4096 keys] stays local; no collectives.

Kernel algebra (per core):
    M^T = Wk^T Wq                       (128x128, one matmul)
    Z   = M x[b]   = (M^T)^T x[b]       [128, 4096]
    S^T[m,q] = sum_i Z[i,m] x[i,q]      -> matmul(lhsT=Z_chunk, rhs=xq), f32r
    P = exp(S^T)                        (ScalarE, PSUM->SBUF, bf16 out;
                                         no max-subtraction: max |S| ~ 67)
    num[o,q] = sum_m V^T[m,o] P[m,q]    -> bf16 PSUM-accumulated matmuls
    den[q]   = sum_m P[m,q]             -> P chunks pre-summed on DVE+GpSimd,
                                           then gpsimd partition_all_reduce
                                           (f32 across-partition sum + bcast)
    out = num * (1/den)                 (reciprocal_approx_fast + multiply)

S^T runs in float32r (1 cycle/row at free dim >= 512, ~19-bit mantissa);
the post-exp path runs in bf16 (linear error only; total ~3e-3).
"""

import numpy as np

import concourse.bass as bass
import concourse.bacc as bacc
import concourse.bass_isa as bass_isa
import concourse.mybir as mybir
from concourse.tile import TileContext
from concourse.bass_utils import run_bass_kernel_spmd
from concourse.masks import make_identity

B, CIN, N = 4, 128, 4096
CKEY, COUT = 64, 128
NCORES = 8
NQ = N // 2            # queries per core
QT = 512               # query supertile (PSUM bank width in f32)
NST = NQ // QT         # 4 supertiles
MC = 128               # key-chunk size (partition dim)
NMC = N // MC          # 32 key chunks
GRP = 3                # key chunks per exp group ([128, 1536] = 3 banks)
WIDE_GROUPS = 9        # groups summed wide (chunks 0..26); rest narrow

F32 = mybir.dt.float32
F32R = mybir.dt.float32r
BF16 = mybir.dt.bfloat16
EXP = mybir.ActivationFunctionType.Exp
ADD = mybir.AluOpType.add


def _build() -> bacc.Bacc:
    nc = bacc.Bacc()
    xq = nc.declare_dram_parameter("xq", [CIN, NQ], F32, isOutput=False)
    xk = nc.declare_dram_parameter("xk", [CIN, N], F32, isOutput=False)
    wq = nc.declare_dram_parameter("wq", [CKEY, CIN], F32, isOutput=False)
    wk = nc.declare_dram_parameter("wk", [CKEY, CIN], F32, isOutput=False)
    wv = nc.declare_dram_parameter("wv", [COUT, CIN], F32, isOutput=False)
    out = nc.declare_dram_parameter("out", [COUT, NQ], F32, isOutput=True)

    with TileContext(nc) as tc:
        with (
            tc.tile_pool(name="big", bufs=1) as big,
            tc.tile_pool(name="ptp", bufs=5) as ptp,
            tc.tile_pool(name="accp", bufs=2) as accp,
            tc.tile_pool(name="outp", bufs=2) as outp,
            tc.tile_pool(name="stp", bufs=2, space="PSUM") as stp,
            tc.tile_pool(name="avp", bufs=2, space="PSUM") as avp,
        ):
            # ---- loads (weights first: they gate the Z chain) ----
            wq_sb = big.tile([CKEY, CIN], F32)
            wk_sb = big.tile([CKEY, CIN], F32)
            wv_sb = big.tile([COUT, CIN], F32)
            nc.sync.dma_start(wq_sb[:], wq[:])
            nc.sync.dma_start(wk_sb[:], wk[:])
            nc.sync.dma_start(wv_sb[:], wv[:])
            xk_sb = big.tile([CIN, N], F32)
            NK4 = N // 4
            for qtr in range(4):
                nc.sync.dma_start(xk_sb[:, qtr * NK4:(qtr + 1) * NK4],
                                  xk[:, qtr * NK4:(qtr + 1) * NK4])
            xq_sb = big.tile([CIN, NQ], F32)
            nc.sync.dma_start(xq_sb[:, :NQ // 2], xq[:, :NQ // 2])
            nc.sync.dma_start(xq_sb[:, NQ // 2:], xq[:, NQ // 2:])

            # ---- critical chain first: padded weights -> M^T -> Z ----
            wq_pad = big.tile([CIN, CIN], F32)
            wk_pad = big.tile([CIN, CIN], F32)
            nc.vector.memset(wq_pad[:], 0.0)
            nc.vector.memset(wk_pad[:], 0.0)
            nc.vector.tensor_copy(wq_pad[:CKEY, :], wq_sb[:])
            nc.vector.tensor_copy(wk_pad[:CKEY, :], wk_sb[:])
            wq_r = big.tile([CIN, CIN], F32R)
            wk_r = big.tile([CIN, CIN], F32R)
            nc.vector.tensor_copy(wq_r[:], wq_pad[:])
            nc.vector.tensor_copy(wk_r[:], wk_pad[:])

            mt_ps = stp.tile([CIN, GRP * QT], F32, tag="ps", name="mt_ps")
            nc.tensor.matmul(mt_ps[:, :CIN], wk_r[:], wq_r[:], start=True, stop=True)
            mt_r = big.tile([CIN, CIN], F32R)
            nc.vector.tensor_copy(mt_r[:], mt_ps[:, :CIN])

            # xk_r quarters (DVE back-to-back) with Z matmuls; Z psum
            # copy-backs go to ScalarE so they don't serialize the casts
            xk_r = big.tile([CIN, N], F32R)
            z_r = big.tile([CIN, N], F32R)
            for qtr in range(4):
                sl = slice(qtr * NK4, (qtr + 1) * NK4)
                nc.vector.tensor_copy(xk_r[:, sl], xk_sb[:, sl])
                zp = stp.tile([CIN, GRP * QT], F32, tag="ps", name="zp")
                nc.tensor.matmul(zp[:, :QT], mt_r[:],
                                 xk_r[:, qtr * NK4: qtr * NK4 + QT],
                                 start=True, stop=True)
                nc.tensor.matmul(zp[:, QT: 2 * QT], mt_r[:],
                                 xk_r[:, qtr * NK4 + QT: (qtr + 1) * NK4],
                                 start=True, stop=True)
                nc.scalar.copy(z_r[:, sl], zp[:, : 2 * QT])

            # queries (first half gates supertile 0) + bf16 keys for V^T
            xq_r = big.tile([CIN, NQ], F32R)
            nc.vector.tensor_copy(xq_r[:, :NQ // 2], xq_sb[:, :NQ // 2])
            xk_bf = big.tile([CIN, N], BF16)
            for qtr in range(4):
                sl = slice(qtr * NK4, (qtr + 1) * NK4)
                nc.vector.tensor_copy(xk_bf[:, sl], xk_sb[:, sl])
            nc.vector.tensor_copy(xq_r[:, NQ // 2:], xq_sb[:, NQ // 2:])

            # ---- V^T path (bf16) ----
            wv_r = big.tile([COUT, CIN], F32R)
            nc.vector.tensor_copy(wv_r[:], wv_sb[:])
            ident_f = big.tile([CIN, CIN], F32)
            make_identity(nc, ident_f[:])
            ident_r = big.tile([CIN, CIN], F32R)
            nc.vector.tensor_copy(ident_r[:], ident_f[:])
            wvt_ps = stp.tile([CIN, GRP * QT], F32, tag="ps", name="wvt_ps")
            nc.tensor.matmul(wvt_ps[:, :CIN], wv_r[:], ident_r[:], start=True, stop=True)
            wvt_bf = big.tile([CIN, COUT], BF16)
            nc.vector.tensor_copy(wvt_bf[:], wvt_ps[:, :CIN])

            # V^T chunk matmuls go through the av pool (free until the
            # first AV accumulation) so they don't rotate the S^T slots
            vt_bf = big.tile([CIN, NMC, MC], BF16)
            for grp in range(NMC // 4):
                vp = avp.tile([CIN, QT], F32, tag="av", name="vp")
                for k in range(4):
                    c = grp * 4 + k
                    nc.tensor.matmul(
                        vp[:, k * MC: (k + 1) * MC],
                        xk_bf[:, c * MC: (c + 1) * MC],
                        wvt_bf[:],
                        start=True, stop=True,
                    )
                nc.vector.tensor_copy(vt_bf[:, grp * 4: (grp + 1) * 4, :],
                                      vp[:, : 4 * MC])

            ones_col_f = big.tile([CIN, 1], F32)
            nc.vector.memset(ones_col_f[:], 1.0)
            ones_col = big.tile([CIN, 1], BF16)
            nc.vector.tensor_copy(ones_col[:], ones_col_f[:])
            ones_row_f = big.tile([1, CIN], F32)
            nc.vector.memset(ones_row_f[:], 1.0)
            ones_row = big.tile([1, CIN], F32R)
            nc.vector.tensor_copy(ones_row[:], ones_row_f[:])

            # ---- main loop over query supertiles ----
            groups = []
            c = 0
            while c < NMC:
                cnt = min(GRP, NMC - c)
                groups.append((c, cnt))
                c += cnt
            n_wide = WIDE_GROUPS * GRP  # chunks summed via wide group adds

            for st in range(NST):
                q0 = st * QT
                xq_st = xq_r[:, q0: q0 + QT]
                av = avp.tile([COUT, QT], F32, tag="av", name="av")
                acc_e = accp.tile([MC, GRP * QT], BF16, name="acc_e", tag="acc_e")
                acc_o = accp.tile([MC, GRP * QT], BF16, name="acc_o", tag="acc_o")
                accn = accp.tile([MC, QT], BF16, name="accn", tag="accn")
                seen = [0, 0]
                for gi, (c0, cnt) in enumerate(groups):
                    ps = stp.tile([MC, GRP * QT], F32, tag="ps", name="ps")
                    for k in range(cnt):
                        nc.tensor.matmul(
                            ps[:, k * QT: (k + 1) * QT],
                            z_r[:, (c0 + k) * MC: (c0 + k + 1) * MC],
                            xq_st, start=True, stop=True)
                    pt = ptp.tile([MC, GRP * QT], BF16, tag="pt", name="pt")
                    nc.scalar.activation(pt[:, : cnt * QT], ps[:, : cnt * QT], EXP)
                    for k in range(cnt):
                        cc = c0 + k
                        nc.tensor.matmul(av[:], vt_bf[:, cc, :],
                                         pt[:, k * QT: (k + 1) * QT],
                                         start=(cc == 0), stop=(cc == NMC - 1))
                    # ---- den accumulation ----
                    if gi < WIDE_GROUPS:
                        # whole-group wide adds on DVE (GpSimd 2-input ops
                        # measured ~4x slower; PE is the bottleneck anyway)
                        par = gi % 2
                        acc = acc_e if par == 0 else acc_o
                        if seen[par] == 0:
                            nc.vector.tensor_copy(acc[:], pt[:])
                        else:
                            nc.vector.tensor_tensor(acc[:], acc[:], pt[:], ADD)
                        seen[par] += 1
                    else:
                        if gi == WIDE_GROUPS:
                            # combine + fold the wide accumulators down to
                            # [128, QT] (runs while the tail groups compute)
                            nc.vector.tensor_tensor(acc_e[:], acc_e[:], acc_o[:], ADD)
                            nc.vector.tensor_tensor(
                                acc_e[:, :QT], acc_e[:, :QT],
                                acc_e[:, QT: 2 * QT], ADD)
                            nc.vector.tensor_tensor(
                                accn[:], acc_e[:, :QT],
                                acc_e[:, 2 * QT: 3 * QT], ADD)
                        for k in range(cnt):
                            nc.vector.tensor_tensor(
                                accn[:], accn[:],
                                pt[:, k * QT: (k + 1) * QT], ADD)

                rb_sb = outp.tile([COUT, QT], F32, name="rb_sb")
                if st < NST - 1:
                    # den: f32 sum over partitions + broadcast on GpSimd
                    # (slow but fully overlapped with the next supertile)
                    den_b = outp.tile([MC, QT], F32, name="den_b")
                    nc.gpsimd.partition_all_reduce(den_b[:], accn[:], MC,
                                                   bass_isa.ReduceOp.add)
                    nc.vector.reciprocal_approx_fast(rb_sb[:], den_b[:])
                else:
                    # last supertile: nothing left to overlap with, so use
                    # the faster PE path through freed PSUM slots
                    dn_ps = stp.tile([MC, GRP * QT], F32, tag="ps", name="dn_ps")
                    nc.tensor.matmul(dn_ps[:1, :QT], ones_col[:], accn[:],
                                     start=True, stop=True)
                    den_r = outp.tile([1, QT], F32R, name="den_r")
                    nc.vector.tensor_copy(den_r[:], dn_ps[:1, :QT])
                    rb_ps = stp.tile([MC, GRP * QT], F32, tag="ps", name="rb_ps")
                    nc.tensor.matmul(rb_ps[:, :QT], ones_row[:], den_r[:],
                                     start=True, stop=True)
                    nc.vector.reciprocal_approx_fast(rb_sb[:], rb_ps[:, :QT])
                o_sb = outp.tile([COUT, QT], F32, name="o_sb")
                nc.vector.tensor_tensor(o_sb[:], av[:], rb_sb[:],
                                        mybir.AluOpType.mult)
                nc.sync.dma_start(out[:, q0: q0 + QT], o_sb[:])

    nc.finalize()
    return nc


_NC_CACHE: list = []
LAST_RESULTS = None


def _get_nc() -> bacc.Bacc:
    if not _NC_CACHE:
        _NC_CACHE.append(_build())
    return _NC_CACHE[0]


def kernel(x, Wq, Wk, Wv, _trace=False):
    global LAST_RESULTS
    x = np.asarray(x, dtype=np.float32)
    wq = np.ascontiguousarray(np.asarray(Wq, dtype=np.float32))
    wk = np.ascontiguousarray(np.asarray(Wk, dtype=np.float32))
    wv = np.ascontiguousarray(np.asarray(Wv, dtype=np.float32))

    nc = _get_nc()
    in_maps = []
    for i in range(NCORES):
        b, h = divmod(i, 2)
        in_maps.append({
            "xq": np.ascontiguousarray(x[b][:, h * NQ: (h + 1) * NQ]),
            "xk": np.ascontiguousarray(x[b]),
            "wq": wq,
            "wk": wk,
            "wv": wv,
        })
    res = run_bass_kernel_spmd(nc, in_maps, core_ids=list(range(NCORES)),
                               trace=_trace)
    LAST_RESULTS = res
    out = np.empty((B, COUT, N), dtype=np.float32)
    for i in range(NCORES):
        b, h = divmod(i, 2)
        out[b][:, h * NQ: (h + 1) * NQ] = res.results[i]["out"]
    return out
